# revision 60
# baseline (speedup 1.0000x reference)
"""Trainium2 Bass kernel for nn_DAELoss_68152541053132.

Contract: kernel(**inputs) takes the FULL inputs (output [512,128,2048] f32,
target [512,128] int) and returns the FULL scalar loss, matching reference().

Strategy (pure data parallel over batch, 8 cores x 64 batches):
  Device per core (memory-bound: one streaming read of 64 MB):
    - ALL x loads on a single HWDGE (sync) queue: one DMA queue preserves
      sequential HBM access (~374-420 GB/s); multi-queue round-robin
      measured ~90 GB/s slower.
    - per batch (position p = one SBUF partition, vocab V=2048 free dim):
        * DVE: 32-wide chunk maxes -> staged and shipped to the host, which
          does the final argmax among chunks + exact refine inside the
          winning chunk (f32 chunk maxes -> exact argmax)
        * ACT: sum_v exp(x) via fused activation accumulator
        * sum_v x (for the label-smoothing mean-logp term): split across
          PE (psum-accumulated fp32 matmuls with w' weights), ACT
          (Copy+accumulator), and GpSimd+DVE (GpSimd folds the row in
          half, DVE finishes) so no engine exceeds the streaming rate.
    - output DMAs ride the otherwise-idle SWDGE (gpsimd) ring.
  Host (cheap [B,S]-sized math):
    - lse = log(sum exp), x[target] gather, argmax from chunk maxes,
      position weights, length penalty, n-gram terms -> total loss.
"""

import numpy as np

B, S, V = 512, 128, 2048
NCORES = 8
BPC = B // NCORES          # batches per core
FC = 32                    # fine-chunk width for argmax
NFC = V // FC              # 64 fine chunks
GRP = 8                    # batches per chunk-max flush group

PAD = 0
LS = 0.1
END_W = 3.0
CHAR_W = 0.2
LEN_P = 0.3
DIFF_MULT = 1.0

_PROGRAM_CACHE = {}


_PAT16 = ("P", "G", "P", "G", "G", "P", "G", "P",
          "G", "P", "G", "G", "P", "G", "P", "G")
# tail: no ACT-copies or DVE full-sums after b60 so no engine's leftover
# work extends far past the end of the stream; b62 is the last PE batch so
# the psum fold overlaps b63
_PAT_LAST8 = ("P", "G", "P", "A", "P", "G", "P", "G")


def _s1_engine(b, bpc=BPC):
    """Which engine computes sum_v x for batch b: 'P' (PE), 'A' (ACT),
    'D' (DVE), 'G' (GpSimd halves the row, DVE finishes).  Shares sized so
    each engine's per-batch load stays under the DMA streaming rate; the
    final 8 batches are arranged so no engine's leftover work extends far
    past the end of the stream."""
    if b >= bpc - 8:
        return _PAT_LAST8[b - (bpc - 8)]
    return _PAT16[b % 16]


def _g_finisher(b, bpc=BPC):
    """For a 'G' batch: which engine finishes the folded 512-wide row.
    2 of 3 go to ACT (it has slack once its copies are gone), 1 of 3 stays
    on DVE; tail G batches stay on DVE to preserve the tuned ending."""
    if b >= bpc - 8:
        return "D"
    g_index = sum(1 for bb in range(b) if _s1_engine(bb, bpc) == "G")
    return "D" if g_index % 3 == 0 else "A"


def _build_program(bpc=BPC):
    """Build the per-core SPMD Bass/Tile program (same program, 8 shards)."""
    from contextlib import ExitStack

    import concourse.bacc as bacc
    import concourse.mybir as mybir
    import concourse.tile as tile

    f32 = mybir.dt.float32
    bf16 = mybir.dt.bfloat16

    nc = bacc.Bacc("TRN2", target_bir_lowering=False)
    x = nc.dram_tensor("x", [bpc, S, V], f32, kind="ExternalInput").ap()
    w = nc.dram_tensor("w", [S, bpc], f32, kind="ExternalInput").ap()
    s2_out = nc.dram_tensor("s2_out", [S, bpc], f32, kind="ExternalOutput").ap()
    # sum_v x for the ACT-share and DVE-share batches (separate tensors so
    # each stage tile has a single writing engine)
    s1a_out = nc.dram_tensor("s1a_out", [S, bpc], f32, kind="ExternalOutput").ap()
    s1d_out = nc.dram_tensor("s1d_out", [S, bpc], f32, kind="ExternalOutput").ap()
    am_out = nc.dram_tensor("am_out", [S, bpc, NFC], f32, kind="ExternalOutput").ap()
    g_out = nc.dram_tensor("g_out", [1, 1], f32, kind="ExternalOutput").ap()

    TPB = 2                    # batches per x tile (2 MB DMAs)
    # the first and last two batches ride their own 1-batch tiles: the
    # first so compute ramps up sooner, the last so the final chains start
    # (and therefore finish) earlier after the stream ends
    chunks = (
        [(0, 1), (1, 1)]
        + [(b0, TPB) for b0 in range(2, bpc - 2, TPB)]
        + [(bpc - 2, 1), (bpc - 1, 1)]
    )
    pe_batches = [b for b in range(bpc) if _s1_engine(b, bpc) == "P"]

    with tile.TileContext(nc) as tc, ExitStack() as ctx:
        xp = ctx.enter_context(tc.tile_pool(name="x", bufs=9))
        xp1 = ctx.enter_context(tc.tile_pool(name="x1", bufs=2))
        ep = ctx.enter_context(tc.tile_pool(name="exp", bufs=2))
        gp = ctx.enter_context(tc.tile_pool(name="ghalf", bufs=3))
        ap_ = ctx.enter_context(tc.tile_pool(name="amst", bufs=2))
        stg = ctx.enter_context(tc.tile_pool(name="stage", bufs=1))
        pp = ctx.enter_context(tc.tile_pool(name="psum", bufs=1, space="PSUM"))

        # stage tiles split in halves so the first half's outputs flush
        # mid-run (tile-granularity dependencies would otherwise hold every
        # output DMA until the final batch)
        HB = bpc // 2
        s2_st = [stg.tile([S, HB], f32, tag=f"s2_st{h}", name=f"s2_st{h}")
                 for h in range(2)]
        s1a_st = [stg.tile([S, HB], f32, tag=f"s1a_st{h}", name=f"s1a_st{h}")
                  for h in range(2)]
        s1d_st = [stg.tile([S, HB], f32, tag=f"s1d_st{h}", name=f"s1d_st{h}")
                  for h in range(2)]
        wt = stg.tile([S, bpc], f32, tag="wt")
        nc.gpsimd.dma_start(wt[:], w[:])

        # PE accumulator for sum_{p,b} w'[p,b] * x[p, v]: [1, 4, 512] f32 =
        # 4 PSUM banks, one long-lived accumulation group per bank.
        psum_acc = pp.tile([1, 4, 512], f32, tag="psum_acc")

        am_stage = None
        pending_hr = []            # deferred DVE finishers for G batches
        pending_hra = []           # deferred ACT finishers for G batches

        def flush_hr(before_b):
            while pending_hr and pending_hr[0][0] < before_b:
                _, gh_t, bb = pending_hr.pop(0)
                nc.vector.tensor_reduce(
                    out=s1d_st[bb // HB][:, bb % HB : bb % HB + 1],
                    in_=gh_t[:],
                    axis=mybir.AxisListType.X,
                    op=mybir.AluOpType.add,
                )

        def flush_hra(before_b):
            while pending_hra and pending_hra[0][0] < before_b:
                _, gh_t, bb = pending_hra.pop(0)
                hd = ep.tile([S, V // 4], bf16, tag="hdump")
                nc.scalar.activation(
                    hd[:],
                    gh_t[:],
                    mybir.ActivationFunctionType.Copy,
                    accum_out=s1a_st[bb // HB][:, bb % HB : bb % HB + 1],
                )

        for b0, tpb in chunks:
            if b0 == bpc // 2 + 2:
                # first-half stages are complete -> flush them mid-stream
                nc.gpsimd.dma_start(s2_out[:, :HB], s2_st[0][:])
                nc.gpsimd.dma_start(s1a_out[:, :HB], s1a_st[0][:])
                nc.gpsimd.dma_start(s1d_out[:, :HB], s1d_st[0][:])
            pool = xp if tpb == TPB else xp1
            xt = pool.tile([S, tpb, V], f32, tag=f"xt{tpb}")
            # single HWDGE queue for the whole stream
            src = x[b0 : b0 + tpb].rearrange("b p v -> p b v")
            nc.sync.dma_start(xt[:], src)

            for j in range(tpb):
                b = b0 + j
                g = b % GRP
                if g == 0:
                    am_stage = ap_.tile([S, GRP, NFC], f32, tag="am_stage")

                # DVE: fine-chunk maxes, staged for the host-side argmax
                nc.vector.tensor_reduce(
                    out=am_stage[:, g, :],
                    in_=xt[:, j, :].rearrange("p (c w) -> p c w", w=FC),
                    axis=mybir.AxisListType.X,
                    op=mybir.AluOpType.max,
                )
                # deferred G-batch half-reduces (gives GpSimd a full batch
                # of slack so the DVE never stalls waiting on it)
                flush_hr(b)

                # ACT: sum_v exp(x) via fused accumulator
                et = ep.tile([S, V], bf16, tag="et")
                nc.scalar.activation(
                    et[:],
                    xt[:, j, :],
                    mybir.ActivationFunctionType.Exp,
                    accum_out=s2_st[b // HB][:, b % HB : b % HB + 1],
                )
                flush_hra(b)

                eng = _s1_engine(b, bpc)
                if eng == "P":
                    # PE: psum_acc[0,c,:] += w[:,b].T @ x[:, c*512:(c+1)*512]
                    # (walrus requires each matmul within one PSUM bank)
                    for c in range(4):
                        nc.tensor.matmul(
                            psum_acc[:, c, :],
                            lhsT=wt[:, b : b + 1],
                            rhs=xt[:, j, c * 512 : (c + 1) * 512],
                            start=(b == pe_batches[0]),
                            stop=(b == pe_batches[-1]),
                        )
                elif eng == "A":
                    # ACT: per-position sum_v x via Copy + accumulator
                    e2 = ep.tile([S, V], bf16, tag="et")
                    nc.scalar.activation(
                        e2[:],
                        xt[:, j, :],
                        mybir.ActivationFunctionType.Copy,
                        accum_out=s1a_st[b // HB][:, b % HB : b % HB + 1],
                    )
                elif eng == "D":
                    # DVE: per-position sum_v x via add-reduce
                    nc.vector.tensor_reduce(
                        out=s1d_st[b // HB][:, b % HB : b % HB + 1],
                        in_=xt[:, j, :],
                        axis=mybir.AxisListType.X,
                        op=mybir.AluOpType.add,
                    )
                else:
                    # GpSimd folds the row twice (2048->1024->512); DVE
                    # finishes the sum at quarter cost (deferred by a
                    # batch, see flush_hr)
                    gh = gp.tile([S, V // 2], f32, tag="gh")
                    nc.gpsimd.tensor_tensor(
                        out=gh[:],
                        in0=xt[:, j, : V // 2],
                        in1=xt[:, j, V // 2 :],
                        op=mybir.AluOpType.add,
                    )
                    gq = gp.tile([S, V // 4], f32, tag="gq")
                    nc.gpsimd.tensor_tensor(
                        out=gq[:],
                        in0=gh[:, : V // 4],
                        in1=gh[:, V // 4 :],
                        op=mybir.AluOpType.add,
                    )
                    if _g_finisher(b, bpc) == "A":
                        pending_hra.append((b, gq, b))
                    else:
                        pending_hr.append((b, gq, b))

                if g == GRP - 1:
                    # flush this group's chunk maxes on the SWDGE ring
                    g0 = b - (GRP - 1)
                    nc.gpsimd.dma_start(
                        am_out[:, g0 : g0 + GRP, :], am_stage[:]
                    )

        # fold the PE accumulator into a scalar on DVE (before the deferred
        # half-reduces so it doesn't queue behind them), then the remaining
        # output flushes ordered by expected readiness
        acc = stg.tile([1, 1], f32, tag="acc")
        nc.vector.tensor_reduce(
            out=acc[:],
            in_=psum_acc[:],
            axis=mybir.AxisListType.XY,
            op=mybir.AluOpType.add,
        )
        flush_hr(bpc + 1)
        flush_hra(bpc + 1)
        nc.gpsimd.dma_start(s1a_out[:, HB:], s1a_st[1][:])
        nc.gpsimd.dma_start(s2_out[:, HB:], s2_st[1][:])
        nc.gpsimd.dma_start(g_out[:], acc[:])
        nc.gpsimd.dma_start(s1d_out[:, HB:], s1d_st[1][:])

    nc.compile()
    return nc


def _get_program(bpc=BPC):
    if bpc not in _PROGRAM_CACHE:
        _PROGRAM_CACHE[bpc] = _build_program(bpc)
    return _PROGRAM_CACHE[bpc]


def _position_weight_matrix(s):
    # Row L-1 holds the position weights for a sequence of length L.
    lf = np.arange(1, s + 1, dtype=np.float32)[:, None]
    jf = np.arange(s, dtype=np.float32)[None, :]
    li = np.arange(1, s + 1)[:, None]
    ji = np.arange(s)[None, :]
    valid = ji < li
    w = np.where(valid, 1.0 + (jf / lf) * 0.5, 1.0).astype(np.float32)
    w = np.where(ji == li - 1, np.float32(END_W * 1.5), w)
    w = np.where((li >= 2) & (ji == li - 2), np.float32(END_W * 1.0), w)
    w = np.where((li >= 3) & (ji == li - 3), np.float32(END_W * 0.8), w)
    mid = (li >= 4) & (ji >= li // 3) & (ji < (2 * li) // 3)
    w = np.where(mid, w * np.float32(1.3), w)
    w = np.where((li <= 4) & valid, w * np.float32(1.2), w)
    return w.astype(np.float32)


def _host_weights(target):
    """bw [B,S] (position weights used in both numerator and denominator)
    and w' = bw * pad_mask (the PE-side reduction weights)."""
    pad_mask = target != PAD
    lens = pad_mask.sum(axis=1)
    wmat = _position_weight_matrix(S)
    rows = wmat[np.clip(lens - 1, 0, S - 1)]
    pos = np.arange(S)[None, :]
    bw = np.where(pos < lens[:, None], rows, np.float32(1.0)).astype(np.float32)
    wprime = np.where(pad_mask, bw, np.float32(0.0)).astype(np.float32)
    return pad_mask, lens, bw, wprime


def _host_finish(output, target, s2, am, g_total):
    """All the cheap [B,S]-sized math, replicating reference() semantics."""
    f64 = np.float64
    pad_mask, lens, bw, _ = _host_weights(target)

    lse = np.log(s2.astype(f64))                      # [B,S]
    bi = np.arange(B)[:, None]
    si = np.arange(S)[None, :]
    x_t = output[bi, si, target.astype(np.int64)].astype(f64)

    # argmax: winning chunk from the device's f32 chunk maxes (exact),
    # then exact refine inside the FC-wide chunk
    ci = am.reshape(B * S, NFC).argmax(axis=1)        # [B*S]
    base = ci.astype(np.int64) * FC
    flat = output.reshape(B * S, V)
    win = flat[np.arange(B * S)[:, None], base[:, None] + np.arange(FC)]
    preds = (base + win.argmax(axis=1)).reshape(B, S)

    # label-smoothed CE with the mean-logp term folded in via g_total:
    #   ce = 0.9*(lse - x_t) + 0.1*(lse - sum_v x / V)   at non-pad, else 0
    #   sum(ce*bw) = sum(bw*mask*(0.9*nll + 0.1*lse)) - 0.1/V * g_total
    ce_part = np.where(pad_mask, 0.9 * (lse - x_t) + 0.1 * lse, 0.0)
    num = (ce_part * bw).sum() - (0.1 / V) * f64(g_total)
    weighted_loss = num / bw.sum(dtype=f64)

    # length penalty
    plen = (preds != PAD).sum(axis=1)
    diff = np.abs(plen.astype(f64) - lens.astype(f64))
    factor = 1.0 + 0.5 * (plen < lens) + 0.3 * (plen <= 3)
    length_pen = LEN_P * (diff * factor).mean()

    # n-gram one-hot MSE (analytic form)
    pb = preds[:, :-1] == preds[:, 1:]
    tb = target[:, :-1] == target[:, 1:]
    mb = pb & tb & (preds[:, :-1] == target[:, :-1])
    bwts = np.where(np.arange(S - 1) >= S - 3, 1.5, 1.0)
    bcnt = pb.astype(f64) + tb.astype(f64) - 2.0 * mb.astype(f64)
    bigram_loss = (bcnt * (bwts**2)).sum() / (B * (S - 1) * V)

    pt = pb[:, :-1] & pb[:, 1:]
    tt = tb[:, :-1] & tb[:, 1:]
    mt = pt & tt & (preds[:, :-2] == target[:, :-2])
    twts = np.where(np.arange(S - 2) >= S - 4, 2.0, 1.0)
    tcnt = pt.astype(f64) + tt.astype(f64) - 2.0 * mt.astype(f64)
    trigram_loss = (tcnt * (twts**2)).sum() / (B * (S - 2) * V)
    any_valid = bool((pad_mask[:, :-2].sum(axis=1) > 0).any())
    ngram_loss = bigram_loss + (1.5 * trigram_loss if any_valid else 0.0)

    total = DIFF_MULT * (
        weighted_loss * 0.7 + length_pen * 0.2 + CHAR_W * ngram_loss * 0.1
    )
    return np.asarray(total, dtype=np.float32)


def _run_device(output, wprime, trace=False):
    """Run the SPMD bass kernel on 8 cores; returns (s2, am, g_total, results)."""
    from concourse.bass_utils import run_bass_kernel_spmd

    nc = _get_program()
    in_maps = []
    for c in range(NCORES):
        shard = output[c * BPC : (c + 1) * BPC]               # view, no copy
        wshard = np.ascontiguousarray(wprime[c * BPC : (c + 1) * BPC].T)
        in_maps.append({"x": shard, "w": wshard})

    res = run_bass_kernel_spmd(nc, in_maps, list(range(NCORES)), trace=trace)

    s2 = np.empty((B, S), np.float32)
    am = np.empty((B, S, NFC), np.float32)
    g_total = 0.0
    a_rows = np.array(
        [b for b in range(BPC)
         if _s1_engine(b) == "A"
         or (_s1_engine(b) == "G" and _g_finisher(b) == "A")],
        dtype=np.int64,
    )
    d_rows = np.array(
        [b for b in range(BPC)
         if _s1_engine(b) == "D"
         or (_s1_engine(b) == "G" and _g_finisher(b) == "D")],
        dtype=np.int64,
    )
    for c in range(NCORES):
        r = res.results[c]
        s2[c * BPC : (c + 1) * BPC] = r["s2_out"].T
        am[c * BPC : (c + 1) * BPC] = r["am_out"].transpose(1, 0, 2)
        # PE batches contribute via the accumulated scalar; the rest via
        # per-position sums weighted on the host
        g_total += r["g_out"].astype(np.float64).sum()
        wp = wprime[c * BPC : (c + 1) * BPC].astype(np.float64)
        s1a = r["s1a_out"].T.astype(np.float64)          # [BPC, S]
        s1d = r["s1d_out"].T.astype(np.float64)
        g_total += (wp[a_rows] * s1a[a_rows]).sum()
        g_total += (wp[d_rows] * s1d[d_rows]).sum()
    return s2, am, g_total, res


def kernel(output, target):
    output = np.asarray(output)
    if output.dtype != np.float32:
        output = output.astype(np.float32)
    target = np.asarray(target)

    _, _, _, wprime = _host_weights(target)
    s2, am, g_total, _ = _run_device(output, wprime)
    return _host_finish(output, target, s2, am, g_total)


# revision 62
# speedup vs baseline: 1.0010x; 1.0010x over previous
"""Trainium2 Bass kernel for nn_DAELoss_68152541053132.

Contract: kernel(**inputs) takes the FULL inputs (output [512,128,2048] f32,
target [512,128] int) and returns the FULL scalar loss, matching reference().

Strategy (pure data parallel over batch, 8 cores x 64 batches):
  Device per core (memory-bound: one streaming read of 64 MB):
    - ALL x loads on a single HWDGE (sync) queue: one DMA queue preserves
      sequential HBM access (~374-420 GB/s); multi-queue round-robin
      measured ~90 GB/s slower.
    - per batch (position p = one SBUF partition, vocab V=2048 free dim):
        * DVE: 32-wide chunk maxes -> staged and shipped to the host, which
          does the final argmax among chunks + exact refine inside the
          winning chunk (f32 chunk maxes -> exact argmax)
        * ACT: sum_v exp(x) via fused activation accumulator
        * sum_v x (for the label-smoothing mean-logp term): split across
          PE (psum-accumulated fp32 matmuls with w' weights), ACT
          (Copy+accumulator), and GpSimd+DVE (GpSimd folds the row in
          half, DVE finishes) so no engine exceeds the streaming rate.
    - output DMAs ride the otherwise-idle SWDGE (gpsimd) ring.
  Host (cheap [B,S]-sized math):
    - lse = log(sum exp), x[target] gather, argmax from chunk maxes,
      position weights, length penalty, n-gram terms -> total loss.
"""

import numpy as np

B, S, V = 512, 128, 2048
NCORES = 8
BPC = B // NCORES          # batches per core
FC = 32                    # fine-chunk width for argmax
NFC = V // FC              # 64 fine chunks
GRP = 8                    # batches per chunk-max flush group

PAD = 0
LS = 0.1
END_W = 3.0
CHAR_W = 0.2
LEN_P = 0.3
DIFF_MULT = 1.0

_PROGRAM_CACHE = {}


_PAT16 = ("P", "G", "P", "G", "G", "P", "G", "P",
          "G", "P", "G", "G", "P", "G", "P", "G")
# tail: no ACT-copies or DVE full-sums after b60 so no engine's leftover
# work extends far past the end of the stream; b62 is the last PE batch so
# the psum fold overlaps b63
_PAT_LAST8 = ("P", "G", "P", "A", "P", "G", "P", "G")


def _s1_engine(b, bpc=BPC):
    """Which engine computes sum_v x for batch b: 'P' (PE), 'A' (ACT),
    'D' (DVE), 'G' (GpSimd halves the row, DVE finishes).  Shares sized so
    each engine's per-batch load stays under the DMA streaming rate; the
    final 8 batches are arranged so no engine's leftover work extends far
    past the end of the stream."""
    if b >= bpc - 8:
        return _PAT_LAST8[b - (bpc - 8)]
    return _PAT16[b % 16]


def _g_finisher(b, bpc=BPC):
    """For a 'G' batch: which engine finishes the folded 512-wide row.
    2 of 3 go to ACT (it has slack once its copies are gone), 1 of 3 stays
    on DVE; tail G batches stay on DVE to preserve the tuned ending."""
    if b >= bpc - 8:
        return "D"
    g_index = sum(1 for bb in range(b) if _s1_engine(bb, bpc) == "G")
    return "D" if g_index % 3 == 0 else "A"


def _build_program(bpc=BPC):
    """Build the per-core SPMD Bass/Tile program (same program, 8 shards)."""
    from contextlib import ExitStack

    import concourse.bacc as bacc
    import concourse.mybir as mybir
    import concourse.tile as tile

    f32 = mybir.dt.float32
    bf16 = mybir.dt.bfloat16

    nc = bacc.Bacc("TRN2", target_bir_lowering=False)
    x = nc.dram_tensor("x", [bpc, S, V], f32, kind="ExternalInput").ap()
    w = nc.dram_tensor("w", [S, bpc], f32, kind="ExternalInput").ap()
    s2_out = nc.dram_tensor("s2_out", [S, bpc], f32, kind="ExternalOutput").ap()
    # sum_v x for the ACT-share and DVE-share batches (separate tensors so
    # each stage tile has a single writing engine)
    s1a_out = nc.dram_tensor("s1a_out", [S, bpc], f32, kind="ExternalOutput").ap()
    s1d_out = nc.dram_tensor("s1d_out", [S, bpc], f32, kind="ExternalOutput").ap()
    am_out = nc.dram_tensor("am_out", [S, bpc, NFC], f32, kind="ExternalOutput").ap()
    g_out = nc.dram_tensor("g_out", [1, 1], f32, kind="ExternalOutput").ap()

    TPB = 2                    # batches per x tile (2 MB DMAs)
    # the first and last two batches ride their own 1-batch tiles: the
    # first so compute ramps up sooner, the last so the final chains start
    # (and therefore finish) earlier after the stream ends
    chunks = (
        [(0, 1), (1, 1)]
        + [(b0, TPB) for b0 in range(2, bpc - 2, TPB)]
        + [(bpc - 2, 1), (bpc - 1, 1)]
    )
    pe_batches = [b for b in range(bpc) if _s1_engine(b, bpc) == "P"]

    with tile.TileContext(nc) as tc, ExitStack() as ctx:
        xp = ctx.enter_context(tc.tile_pool(name="x", bufs=9))
        xp1 = ctx.enter_context(tc.tile_pool(name="x1", bufs=2))
        ep = ctx.enter_context(tc.tile_pool(name="exp", bufs=2))
        gp = ctx.enter_context(tc.tile_pool(name="ghalf", bufs=3))
        ap_ = ctx.enter_context(tc.tile_pool(name="amst", bufs=2))
        stg = ctx.enter_context(tc.tile_pool(name="stage", bufs=1))
        pp = ctx.enter_context(tc.tile_pool(name="psum", bufs=1, space="PSUM"))

        # stage tiles split in halves so the first half's outputs flush
        # mid-run (tile-granularity dependencies would otherwise hold every
        # output DMA until the final batch)
        HB = bpc // 2
        s2_st = [stg.tile([S, HB], f32, tag=f"s2_st{h}", name=f"s2_st{h}")
                 for h in range(2)]
        s1a_st = [stg.tile([S, HB], f32, tag=f"s1a_st{h}", name=f"s1a_st{h}")
                  for h in range(2)]
        s1d_st = [stg.tile([S, HB], f32, tag=f"s1d_st{h}", name=f"s1d_st{h}")
                  for h in range(2)]
        wt = stg.tile([S, bpc], f32, tag="wt")
        nc.gpsimd.dma_start(wt[:], w[:])

        # PE accumulator for sum_{p,b} w'[p,b] * x[p, v]: [1, 4, 512] f32 =
        # 4 PSUM banks, one long-lived accumulation group per bank.
        psum_acc = pp.tile([1, 4, 512], f32, tag="psum_acc")

        am_stage = None
        pending_hr = []            # deferred DVE finishers for G batches
        pending_hra = []           # deferred ACT finishers for G batches

        def flush_hr(before_b):
            while pending_hr and pending_hr[0][0] < before_b:
                _, gh_t, bb = pending_hr.pop(0)
                nc.vector.tensor_reduce(
                    out=s1d_st[bb // HB][:, bb % HB : bb % HB + 1],
                    in_=gh_t[:],
                    axis=mybir.AxisListType.X,
                    op=mybir.AluOpType.add,
                )

        def flush_hra(before_b):
            while pending_hra and pending_hra[0][0] < before_b:
                _, gh_t, bb = pending_hra.pop(0)
                hd = ep.tile([S, V // 4], bf16, tag="hdump")
                nc.scalar.activation(
                    hd[:],
                    gh_t[:],
                    mybir.ActivationFunctionType.Copy,
                    accum_out=s1a_st[bb // HB][:, bb % HB : bb % HB + 1],
                )

        for b0, tpb in chunks:
            if b0 == bpc // 2 + 2:
                # first-half stages are complete -> flush them mid-stream
                nc.gpsimd.dma_start(s2_out[:, :HB], s2_st[0][:])
                nc.gpsimd.dma_start(s1a_out[:, :HB], s1a_st[0][:])
                nc.gpsimd.dma_start(s1d_out[:, :HB], s1d_st[0][:])
            pool = xp if tpb == TPB else xp1
            xt = pool.tile([S, tpb, V], f32, tag=f"xt{tpb}")
            # single HWDGE queue for the whole stream
            src = x[b0 : b0 + tpb].rearrange("b p v -> p b v")
            nc.sync.dma_start(xt[:], src)

            for j in range(tpb):
                b = b0 + j
                g = b % GRP
                if g == 0:
                    am_stage = ap_.tile([S, GRP, NFC], f32, tag="am_stage")

                # DVE: fine-chunk maxes, staged for the host-side argmax
                nc.vector.tensor_reduce(
                    out=am_stage[:, g, :],
                    in_=xt[:, j, :].rearrange("p (c w) -> p c w", w=FC),
                    axis=mybir.AxisListType.X,
                    op=mybir.AluOpType.max,
                )
                # deferred G-batch half-reduces (gives GpSimd a full batch
                # of slack so the DVE never stalls waiting on it)
                flush_hr(b)

                # ACT: sum_v exp(x) via fused accumulator
                et = ep.tile([S, V], bf16, tag="et")
                nc.scalar.activation(
                    et[:],
                    xt[:, j, :],
                    mybir.ActivationFunctionType.Exp,
                    accum_out=s2_st[b // HB][:, b % HB : b % HB + 1],
                )
                flush_hra(b)

                eng = _s1_engine(b, bpc)
                if eng == "P":
                    # PE: psum_acc[0,c,:] += w[:,b].T @ x[:, c*512:(c+1)*512]
                    # (walrus requires each matmul within one PSUM bank)
                    for c in range(4):
                        nc.tensor.matmul(
                            psum_acc[:, c, :],
                            lhsT=wt[:, b : b + 1],
                            rhs=xt[:, j, c * 512 : (c + 1) * 512],
                            start=(b == pe_batches[0]),
                            stop=(b == pe_batches[-1]),
                        )
                elif eng == "A":
                    # ACT: per-position sum_v x via Copy + accumulator
                    e2 = ep.tile([S, V], bf16, tag="et")
                    nc.scalar.activation(
                        e2[:],
                        xt[:, j, :],
                        mybir.ActivationFunctionType.Copy,
                        accum_out=s1a_st[b // HB][:, b % HB : b % HB + 1],
                    )
                elif eng == "D":
                    # DVE: per-position sum_v x via add-reduce
                    nc.vector.tensor_reduce(
                        out=s1d_st[b // HB][:, b % HB : b % HB + 1],
                        in_=xt[:, j, :],
                        axis=mybir.AxisListType.X,
                        op=mybir.AluOpType.add,
                    )
                else:
                    # GpSimd folds the row twice (2048->1024->512); DVE
                    # finishes the sum at quarter cost (deferred by a
                    # batch, see flush_hr)
                    gh = gp.tile([S, V // 2], f32, tag="gh")
                    nc.gpsimd.tensor_tensor(
                        out=gh[:],
                        in0=xt[:, j, : V // 2],
                        in1=xt[:, j, V // 2 :],
                        op=mybir.AluOpType.add,
                    )
                    gq = gp.tile([S, V // 4], f32, tag="gq")
                    nc.gpsimd.tensor_tensor(
                        out=gq[:],
                        in0=gh[:, : V // 4],
                        in1=gh[:, V // 4 :],
                        op=mybir.AluOpType.add,
                    )
                    if _g_finisher(b, bpc) == "A":
                        pending_hra.append((b, gq, b))
                    else:
                        pending_hr.append((b, gq, b))

                if g == GRP - 1:
                    # flush this group's chunk maxes on the SWDGE ring
                    g0 = b - (GRP - 1)
                    nc.gpsimd.dma_start(
                        am_out[:, g0 : g0 + GRP, :], am_stage[:]
                    )

        # fold the PE accumulator into a scalar on DVE (before the deferred
        # half-reduces so it doesn't queue behind them), then the remaining
        # output flushes ordered by expected readiness
        acc = stg.tile([1, 1], f32, tag="acc")
        nc.vector.tensor_reduce(
            out=acc[:],
            in_=psum_acc[:],
            axis=mybir.AxisListType.XY,
            op=mybir.AluOpType.add,
        )
        flush_hr(bpc + 1)
        flush_hra(bpc + 1)
        nc.gpsimd.dma_start(s1a_out[:, HB:], s1a_st[1][:])
        nc.gpsimd.dma_start(s2_out[:, HB:], s2_st[1][:])
        nc.gpsimd.dma_start(g_out[:], acc[:])
        nc.gpsimd.dma_start(s1d_out[:, HB:], s1d_st[1][:])

    nc.compile()
    return nc


def _get_program(bpc=BPC):
    if bpc not in _PROGRAM_CACHE:
        _PROGRAM_CACHE[bpc] = _build_program(bpc)
    return _PROGRAM_CACHE[bpc]


def _position_weight_matrix(s):
    # Row L-1 holds the position weights for a sequence of length L.
    lf = np.arange(1, s + 1, dtype=np.float32)[:, None]
    jf = np.arange(s, dtype=np.float32)[None, :]
    li = np.arange(1, s + 1)[:, None]
    ji = np.arange(s)[None, :]
    valid = ji < li
    w = np.where(valid, 1.0 + (jf / lf) * 0.5, 1.0).astype(np.float32)
    w = np.where(ji == li - 1, np.float32(END_W * 1.5), w)
    w = np.where((li >= 2) & (ji == li - 2), np.float32(END_W * 1.0), w)
    w = np.where((li >= 3) & (ji == li - 3), np.float32(END_W * 0.8), w)
    mid = (li >= 4) & (ji >= li // 3) & (ji < (2 * li) // 3)
    w = np.where(mid, w * np.float32(1.3), w)
    w = np.where((li <= 4) & valid, w * np.float32(1.2), w)
    return w.astype(np.float32)


def _host_weights(target):
    """bw [B,S] (position weights used in both numerator and denominator)
    and w' = bw * pad_mask (the PE-side reduction weights)."""
    pad_mask = target != PAD
    lens = pad_mask.sum(axis=1)
    wmat = _position_weight_matrix(S)
    rows = wmat[np.clip(lens - 1, 0, S - 1)]
    pos = np.arange(S)[None, :]
    bw = np.where(pos < lens[:, None], rows, np.float32(1.0)).astype(np.float32)
    wprime = np.where(pad_mask, bw, np.float32(0.0)).astype(np.float32)
    return pad_mask, lens, bw, wprime


def _host_finish(output, target, s2, am, g_total):
    """All the cheap [B,S]-sized math, replicating reference() semantics."""
    f64 = np.float64
    pad_mask, lens, bw, _ = _host_weights(target)

    lse = np.log(s2.astype(f64))                      # [B,S]
    bi = np.arange(B)[:, None]
    si = np.arange(S)[None, :]
    x_t = output[bi, si, target.astype(np.int64)].astype(f64)

    # argmax: winning chunk from the device's f32 chunk maxes (exact),
    # then exact refine inside the FC-wide chunk
    ci = am.reshape(B * S, NFC).argmax(axis=1)        # [B*S]
    base = ci.astype(np.int64) * FC
    flat = output.reshape(B * S, V)
    win = flat[np.arange(B * S)[:, None], base[:, None] + np.arange(FC)]
    preds = (base + win.argmax(axis=1)).reshape(B, S)

    # label-smoothed CE with the mean-logp term folded in via g_total:
    #   ce = 0.9*(lse - x_t) + 0.1*(lse - sum_v x / V)   at non-pad, else 0
    #   sum(ce*bw) = sum(bw*mask*(0.9*nll + 0.1*lse)) - 0.1/V * g_total
    ce_part = np.where(pad_mask, 0.9 * (lse - x_t) + 0.1 * lse, 0.0)
    num = (ce_part * bw).sum() - (0.1 / V) * f64(g_total)
    weighted_loss = num / bw.sum(dtype=f64)

    # length penalty
    plen = (preds != PAD).sum(axis=1)
    diff = np.abs(plen.astype(f64) - lens.astype(f64))
    factor = 1.0 + 0.5 * (plen < lens) + 0.3 * (plen <= 3)
    length_pen = LEN_P * (diff * factor).mean()

    # n-gram one-hot MSE (analytic form)
    pb = preds[:, :-1] == preds[:, 1:]
    tb = target[:, :-1] == target[:, 1:]
    mb = pb & tb & (preds[:, :-1] == target[:, :-1])
    bwts = np.where(np.arange(S - 1) >= S - 3, 1.5, 1.0)
    bcnt = pb.astype(f64) + tb.astype(f64) - 2.0 * mb.astype(f64)
    bigram_loss = (bcnt * (bwts**2)).sum() / (B * (S - 1) * V)

    pt = pb[:, :-1] & pb[:, 1:]
    tt = tb[:, :-1] & tb[:, 1:]
    mt = pt & tt & (preds[:, :-2] == target[:, :-2])
    twts = np.where(np.arange(S - 2) >= S - 4, 2.0, 1.0)
    tcnt = pt.astype(f64) + tt.astype(f64) - 2.0 * mt.astype(f64)
    trigram_loss = (tcnt * (twts**2)).sum() / (B * (S - 2) * V)
    any_valid = bool((pad_mask[:, :-2].sum(axis=1) > 0).any())
    ngram_loss = bigram_loss + (1.5 * trigram_loss if any_valid else 0.0)

    total = DIFF_MULT * (
        weighted_loss * 0.7 + length_pen * 0.2 + CHAR_W * ngram_loss * 0.1
    )
    return np.asarray(total, dtype=np.float32)


def _run_device(output, wprime, trace=False):
    """Run the SPMD bass kernel on 8 cores; returns (s2, am, g_total, results)."""
    from concourse.bass_utils import run_bass_kernel_spmd

    nc = _get_program()
    in_maps = []
    for c in range(NCORES):
        shard = output[c * BPC : (c + 1) * BPC]               # view, no copy
        wshard = np.ascontiguousarray(wprime[c * BPC : (c + 1) * BPC].T)
        in_maps.append({"x": shard, "w": wshard})

    res = run_bass_kernel_spmd(nc, in_maps, list(range(NCORES)), trace=trace)

    s2 = np.empty((B, S), np.float32)
    am = np.empty((B, S, NFC), np.float32)
    g_total = 0.0
    a_rows = np.array(
        [b for b in range(BPC)
         if _s1_engine(b) == "A"
         or (_s1_engine(b) == "G" and _g_finisher(b) == "A")],
        dtype=np.int64,
    )
    d_rows = np.array(
        [b for b in range(BPC)
         if _s1_engine(b) == "D"
         or (_s1_engine(b) == "G" and _g_finisher(b) == "D")],
        dtype=np.int64,
    )
    for c in range(NCORES):
        r = res.results[c]
        s2[c * BPC : (c + 1) * BPC] = r["s2_out"].T
        am[c * BPC : (c + 1) * BPC] = r["am_out"].transpose(1, 0, 2)
        # PE batches contribute via the accumulated scalar; the rest via
        # per-position sums weighted on the host
        g_total += r["g_out"].astype(np.float64).sum()
        wp = wprime[c * BPC : (c + 1) * BPC].astype(np.float64)
        s1a = r["s1a_out"].T.astype(np.float64)          # [BPC, S]
        s1d = r["s1d_out"].T.astype(np.float64)
        g_total += (wp[a_rows] * s1a[a_rows]).sum()
        g_total += (wp[d_rows] * s1d[d_rows]).sum()
    return s2, am, g_total, res


def kernel(output, target):
    output = np.asarray(output)
    if output.dtype != np.float32:
        output = output.astype(np.float32)
    target = np.asarray(target)

    _, _, _, wprime = _host_weights(target)
    s2, am, g_total, _ = _run_device(output, wprime)
    return _host_finish(output, target, s2, am, g_total)
